# revision 27
# baseline (speedup 1.0000x reference)
"""CNN(ResNet50)+Mamba classifier on 8 trn2 NeuronCores.

Data-parallel over batch: 16 images -> 2 per core. Each core runs the full
network on its 2 images; host concatenates [2,1000] logits -> [16,1000].
"""
import os
import sys
import math

sys.path.insert(0, "/opt/trn_rl_repo")

import numpy as np
import ml_dtypes

import concourse.bass as bass
import concourse.bacc as bacc
import concourse.mybir as mybir
import concourse.tile as tile
from concourse.bass_utils import run_bass_kernel_spmd

f32 = mybir.dt.float32
bf16 = mybir.dt.bfloat16
fp16 = mybir.dt.float16
AF = mybir.ActivationFunctionType
OP = mybir.AluOpType
AX = mybir.AxisListType
BF = np.float16  # CNN device dtype (fp16: 11-bit mantissa, range ok here)

N_CORES = 8
IMGS_PER_CORE = 2
D_MODEL = 256
D_INNER = 512
D_STATE = 16
DT_RANK = 16
D_CONV = 4
NUM_CLASSES = 1000
L_TOK = 49

# (name, mid, cout, nblocks, stride, Hin, Hout)
STAGES = [
    ("s1", 64, 256, 3, 1, 56, 56),
    ("s2", 128, 512, 4, 2, 56, 28),
    ("s3", 256, 1024, 6, 2, 28, 14),
    ("s4", 512, 2048, 3, 2, 14, 7),
]

K_UPTO = os.environ.get("K_UPTO", "full")  # stem|s1|s2|s3|s4|patch|m0|m1|full
DBG_SHAPES = {}  # filled at build: name -> shape of debug output


def _cdiv(a, b):
    return (a + b - 1) // b


# ----------------------------------------------------------------------------
# Host-side weight preparation
# ----------------------------------------------------------------------------

def _conv_lhsT(w, scale=None):
    """w [oc, ic, kh, kw] (optionally * scale[oc]) -> [Kch, 128, taps*oc] bf16.

    Row order (ic, kh, kw); col order (kh, kw, oc) i.e. tap-major.
    """
    w = np.asarray(w, np.float32)
    if scale is not None:
        w = w * np.asarray(scale, np.float32)[:, None, None, None]
    oc, ic, kh, kw = w.shape
    t = w.transpose(1, 2, 3, 0).reshape(ic, kh * kw * oc)
    Kch = _cdiv(ic, 128)
    if ic % 128:
        t = np.pad(t, ((0, Kch * 128 - ic), (0, 0)))
    return np.ascontiguousarray(t.reshape(Kch, 128, -1)).astype(BF)


def _bias_arr(b, cout):
    b = np.asarray(b, np.float32)
    Mch = _cdiv(cout, 128)
    if cout % 128:
        b = np.pad(b, (0, Mch * 128 - cout))
    return np.ascontiguousarray(b.reshape(Mch, 128).T).astype(np.float32)


def _prep_weights(params):
    ins = {}

    def jn(a):
        return np.asarray(a, np.float32)

    # stem: rows (ic, kh, kw)=147 must match host im2col order
    sw = jn(params["stem_w"]) * jn(params["stem_s"])[:, None, None, None]
    st = sw.transpose(1, 2, 3, 0).reshape(147, 64)
    ins["W_stem"] = np.ascontiguousarray(
        np.pad(st, ((0, 109), (0, 0))).reshape(2, 128, 64)).astype(BF)
    ins["B_stem"] = np.ascontiguousarray(
        np.tile(jn(params["stem_b"]), 2)[:, None]).astype(np.float32)

    for (sname, mid, cout, nb, stride, Hin, Hout), blocks in zip(STAGES, params["stages"]):
        for j, p in enumerate(blocks):
            pre = f"{sname}b{j}"
            ins[f"W_{pre}_1"] = _conv_lhsT(jn(p["w1"]), jn(p["s1"]))
            ins[f"B_{pre}_1"] = _bias_arr(jn(p["b1"]), mid)
            ins[f"W_{pre}_2"] = _conv_lhsT(jn(p["w2"]), jn(p["s2"]))
            ins[f"B_{pre}_2"] = _bias_arr(jn(p["b2"]), mid)
            ins[f"W_{pre}_3"] = _conv_lhsT(jn(p["w3"]), jn(p["s3"]))
            b3 = jn(p["b3"])
            if j == 0:
                ins[f"W_{pre}_dw"] = _conv_lhsT(jn(p["dw"]), jn(p["ds"]))
                b3 = b3 + jn(p["db"])
            ins[f"B_{pre}_3"] = _bias_arr(b3, cout)

    ins["W_patch"] = _conv_lhsT(jn(params["patch_w"]))
    ins["B_patch"] = _bias_arr(jn(params["patch_b"]), D_MODEL)

    for i, mp in enumerate(params["mamba"]):
        in_w = jn(mp["in_w"])          # [1024, 256]
        ins[f"W_m{i}_in"] = np.ascontiguousarray(
            in_w.T.reshape(2, 128, 2 * D_INNER)).astype(np.float32)
        cw = jn(mp["conv_w"])[:, 0, :]  # [512, 4]
        ins[f"W_m{i}_conv"] = np.ascontiguousarray(
            cw.reshape(4, 128, 4).transpose(1, 0, 2).reshape(128, 16)).astype(np.float32)
        ins[f"B_m{i}_conv"] = np.ascontiguousarray(
            jn(mp["conv_b"]).reshape(4, 128).T).astype(np.float32)
        xp = jn(mp["xproj_w"])          # [48, 512]
        ins[f"W_m{i}_xp"] = np.ascontiguousarray(
            xp.T.reshape(4, 128, DT_RANK + 2 * D_STATE)).astype(np.float32)
        dtw = jn(mp["dt_w"])            # [512, 16]
        ins[f"W_m{i}_dt"] = np.ascontiguousarray(dtw.T)[None].astype(np.float32)
        ins[f"B_m{i}_dt"] = np.ascontiguousarray(
            jn(mp["dt_b"]).reshape(4, 128).T).astype(np.float32)
        A = -np.exp(jn(mp["A_log"]))    # [512, 16]
        ins[f"A_m{i}"] = np.ascontiguousarray(
            A.reshape(4, 128, 16).transpose(1, 0, 2).reshape(128, 64)).astype(np.float32)
        ins[f"D_m{i}"] = np.ascontiguousarray(
            jn(mp["D"]).reshape(4, 128).T).astype(np.float32)
        ow = jn(mp["out_w"])            # [256, 512]
        ins[f"W_m{i}_out"] = np.ascontiguousarray(
            ow.T.reshape(4, 128, D_MODEL)).astype(np.float32)

    cw = jn(params["cls_w"]) / float(L_TOK)   # fold mean-pool 1/49
    cwT = np.zeros((D_MODEL, 1024), np.float32)
    cwT[:, :NUM_CLASSES] = cw.T
    ins["W_cls"] = np.ascontiguousarray(cwT.reshape(2, 128, 1024)).astype(np.float32)
    ins["B_cls"] = _bias_arr(np.pad(jn(params["cls_b"]), (0, 24)), 1024)

    ins["EYE"] = np.eye(128, dtype=np.float32).astype(np.float16)
    return ins


def _im2col_stem(x2):
    """x2 [2,3,224,224] f32 -> [2, 147, 12544] bf16 (rows ic*49+kh*7+kw)."""
    xp = np.pad(np.asarray(x2, np.float32), ((0, 0), (0, 0), (3, 3), (3, 3)))
    s = xp.strides
    from numpy.lib.stride_tricks import as_strided
    v = as_strided(xp, shape=(2, 3, 7, 7, 112, 112),
                   strides=(s[0], s[1], s[2], s[3], 2 * s[2], 2 * s[3]))
    return np.ascontiguousarray(v.reshape(2, 147, 12544)).astype(BF)


# ----------------------------------------------------------------------------
# Device kernel builder
# ----------------------------------------------------------------------------

class ActT:
    """SBUF activation tensor [P, (img, Cch, Hs, Ws)] with optional 1-px pad."""

    def __init__(self, t, C, H, W, pad):
        self.t, self.C, self.H, self.W, self.pad = t, C, H, W, pad
        self.P = min(C, 128)
        self.Cch = _cdiv(C, 128)
        self.Hs, self.Ws = H + 2 * pad, W + 2 * pad

    @staticmethod
    def alloc(pool, C, H, W, pad, dtype=fp16, tag=None):
        P = min(C, 128)
        Cch = _cdiv(C, 128)
        t = pool.tile([P, IMGS_PER_CORE * Cch * (H + 2 * pad) * (W + 2 * pad)],
                      dtype, tag=tag)
        return ActT(t, C, H, W, pad)

    def view(self):
        return self.t[:].rearrange("p (i c h w) -> p i c h w",
                                   i=IMGS_PER_CORE, c=self.Cch, h=self.Hs, w=self.Ws)


class Builder:
    def __init__(self, nc, tc, ctx_pools):
        self.nc = nc
        self.tc = tc
        self.pools = ctx_pools
        self.dram = {}
        self.act_cost = 0.0   # running balance for epilogue engine choice
        self.dve_cost = 0.0
        self.eye = None
        self.wcache = {}

    def din(self, name):
        return self.dram[name]

    # -- weight/bias loading ------------------------------------------------
    def load_w(self, pool, name, kc):
        key = (name, kc)
        if key in self.wcache:
            return self.wcache[key]
        arr = self.din(name)            # [Kch, P, cols]
        cols = arr.shape[2]
        bufs = pool._bucket_bufs.get(cols)
        wdt = _INPUT_SHAPES[name][1]
        t = pool.tile([arr.shape[1], cols], wdt,
                      tag=f"{pool._wtag}{cols}", bufs=bufs, name="wt")
        self.nc.sync.dma_start(t[:], arr[kc])
        self.wcache[key] = t
        return t

    def load_bias(self, pool, name):
        key = (name, "b")
        if key in self.wcache:
            return self.wcache[key]
        arr = self.din(name)            # [128, Mch]
        t = pool.tile([128, arr.shape[1]], f32, tag="bias")
        self.nc.sync.dma_start(t[:], arr[:])
        self.wcache[key] = t
        return t

    # -- epilogue -----------------------------------------------------------
    def epilogue(self, psum_ap, dst_ap, bias_ap, relu):
        n = psum_ap.free_size()
        cost_act = (172.0 + n) / 1.2
        cost_dve = (120.0 + n) / 0.96
        if self.act_cost + cost_act <= self.dve_cost + cost_dve:
            self.act_cost += cost_act
            self.nc.scalar.activation(dst_ap, psum_ap,
                                      AF.Relu if relu else AF.Identity,
                                      bias=bias_ap)
        else:
            self.dve_cost += cost_dve
            if relu:
                self.nc.vector.tensor_scalar(dst_ap, psum_ap, bias_ap, 0.0,
                                             op0=OP.add, op1=OP.max)
            else:
                self.nc.vector.tensor_scalar_add(dst_ap, psum_ap, bias_ap)

    # -- generic conv -------------------------------------------------------
    def conv(self, wpool, pspool, name, src, dst, Cout, ksize, stride,
             relu=True, dw_src=None, dw_name=None, dw_stride=1, res_src=None):
        """src/dst: ActT. Output grid (Ho, Wo) = dst interior.

        sources: main conv (from src), optional dw conv (1x1, stride, from
        dw_src) accumulated into same psum, optional identity residual
        (res_src) accumulated via EYE matmul.
        """
        nc = self.nc
        Ho, Wo = dst.H, dst.W
        Cin = src.C
        Kch = _cdiv(Cin, 128)
        Mch = _cdiv(Cout, 128)
        taps = ksize * ksize
        assert (ksize == 1 and src.pad == 0) or (ksize == 3 and src.pad == 1)

        # n-tiles
        if 2 * Ho * Wo <= 512:
            ntiles = [None]               # fused both images
        else:
            rpt = max(1, 448 // Wo)
            ntiles = [(i, r, min(r + rpt, Ho))
                      for i in range(IMGS_PER_CORE) for r in range(0, Ho, rpt)]

        sources = [(name, src, ksize, stride, Kch, taps)]
        if dw_src is not None:
            sources.append((dw_name, dw_src, 1, dw_stride,
                            _cdiv(dw_src.C, 128), 1))
        n_mm_extra = 1 if res_src is not None else 0
        mm_per_psum = sum(kc * tp for (_, _, _, _, kc, tp) in sources) + n_mm_extra

        bias_sb = self.load_bias(wpool, "B_" + name[2:])

        def rhs_ap(asrc, kc, kh, kw, s, nt, ho, wo):
            v = asrc.view()
            if nt is None:
                return v[:, :, kc, kh:kh + s * ho:s, kw:kw + s * wo:s]
            img, r0, r1 = nt
            return v[:, img, kc, kh + s * r0:kh + s * r1:s, kw:kw + s * wo:s]

        def dst_ap(m, nt):
            v = dst.view()
            p = dst.pad
            if nt is None:
                return v[:, :, m, p:p + Ho, p:p + Wo]
            img, r0, r1 = nt
            return v[:, img, m, p + r0:p + r1, p:p + Wo]

        # preload all weights for this conv (resident; bucketed slot bufs)
        for (wn, _, _, _, kcn, _) in sources:
            for kc in range(kcn):
                self.load_w(wpool, wn, kc)

        # contiguous accumulation per psum tile (interleaved groups hang HW)
        for m in range(Mch):
            mc = min(128, Cout - m * 128)
            for nt in ntiles:
                if nt is None:
                    N = 2 * Ho * Wo
                else:
                    _, r0, r1 = nt
                    N = (r1 - r0) * Wo
                ps = pspool.tile([mc, N], f32, tag="ps", name="psc")
                i = 0
                for (wn, asrc, ksz, s, kcn, tp) in sources:
                    for kc in range(kcn):
                        wt = self.load_w(wpool, wn, kc)
                        kp = min(128, asrc.C - kc * 128)
                        for kh in range(ksz):
                            for kw in range(ksz):
                                tp_i = kh * ksz + kw
                                nc.tensor.matmul(
                                    ps[:],
                                    wt[:kp, tp_i * Cout + m * 128:
                                       tp_i * Cout + m * 128 + mc],
                                    rhs_ap(asrc, kc, kh, kw, s, nt, Ho, Wo),
                                    start=(i == 0),
                                    stop=(i == mm_per_psum - 1))
                                i += 1
                if res_src is not None:
                    nc.tensor.matmul(
                        ps[:], self.eye[:, :mc],
                        rhs_ap(res_src, m, 0, 0, 1, nt, Ho, Wo),
                        start=(i == 0), stop=(i == mm_per_psum - 1))
                    i += 1
                self.epilogue(ps[:], dst_ap(m, nt), bias_sb[:mc, m:m + 1], relu)
        for (wn, _, _, _, kcn, _) in sources:
            for kc in range(kcn):
                self.wcache.pop((wn, kc), None)


def build(nc, debug_tap=None):
    """Emit the full kernel. Returns nothing; declares 'out' (and 'dbg')."""
    ins_shapes = _INPUT_SHAPES
    dram = {}
    for name, (shape, dt) in ins_shapes.items():
        dram[name] = nc.declare_dram_parameter(name, list(shape), dt,
                                               isOutput=False)[:]
    out_d = nc.declare_dram_parameter("out", [IMGS_PER_CORE, NUM_CLASSES], f32,
                                      isOutput=True)[:]
    dbg_d = None
    if debug_tap is not None:
        DBG_SHAPES[debug_tap[0]] = debug_tap[1]
        dbg_d = nc.declare_dram_parameter("dbg", list(debug_tap[1]), f32,
                                          isOutput=True)[:]

    with tile.TileContext(nc) as tc:
        import contextlib
        with contextlib.ExitStack() as ctx:
            b = Builder(nc, tc, None)
            b.dram = dram

            pspool = ctx.enter_context(tc.tile_pool(name="ps", bufs=8, space="PSUM"))
            cpool = ctx.enter_context(tc.tile_pool(name="const", bufs=1))
            hand = ctx.enter_context(tc.tile_pool(name="hand", bufs=2))
            drpool = ctx.enter_context(tc.tile_pool(name="dr", bufs=2, space="DRAM"))

            eye = cpool.tile([128, 128], fp16, tag="eye")
            nc.sync.dma_start(eye[:], dram["EYE"])
            b.eye = eye

            def dbg_dump(act, upto_name):
                """DMA an ActT (full tile, f32-cast via gpsimd) to dbg."""
                if dbg_d is None or debug_tap[0] != upto_name:
                    return False
                nc.gpsimd.dma_start(dbg_d, act.t[:act.P])
                return True

            # ---------------- stem + maxpool ----------------
            with tc.tile_pool(name="stem", bufs=1) as stpool, \
                 tc.tile_pool(name="imc", bufs=6) as imcpool, \
                 tc.tile_pool(name="wstem", bufs=3) as wstem:
                wstem._wtag = "wstem"
                wstem._bucket_bufs = {64: 3}
                stem_w0 = b.load_w(wstem, "W_stem", 0)
                stem_w1 = b.load_w(wstem, "W_stem", 1)
                stem_b2 = b.load_bias(wstem, "B_stem")

                # stem_pad packs both images on partitions: p = img*64 + ch
                stem_pad = stpool.tile([128, 114 * 114], fp16, tag="stem_pad")
                v = stem_pad[:].rearrange("p (h w) -> p h w", h=114)
                nc.gpsimd.memset(v[:, 0:114:113, :], 0.0)
                nc.gpsimd.memset(v[:, :, 0:114:113], 0.0)

                imc = dram["X_imc"]  # [2, 147, 12544]
                for nt in range(28):            # 4 rows of 112 per tile
                    c0 = nt * 448
                    ps = pspool.tile([128, 448], f32, tag="ps")
                    for img in range(IMGS_PER_CORE):
                        t0 = imcpool.tile([128, 448], fp16, tag="imc0")
                        nc.sync.dma_start(t0[:], imc[img, 0:128, c0:c0 + 448])
                        t1 = imcpool.tile([19, 448], fp16, tag="imc1")
                        nc.sync.dma_start(t1[:], imc[img, 128:147, c0:c0 + 448])
                        nc.tensor.matmul(ps[img * 64:img * 64 + 64, :],
                                         stem_w0[:, :64], t0[:],
                                         start=True, stop=False)
                        nc.tensor.matmul(ps[img * 64:img * 64 + 64, :],
                                         stem_w1[:19, :64], t1[:],
                                         start=False, stop=True)
                    r0 = nt * 4
                    b.epilogue(ps[:], v[:, 1 + r0:5 + r0, 1:113],
                               stem_b2[:, 0:1], True)

                # maxpool 3x3/2 (both images via partition packing)
                colmax = stpool.tile([128, 114 * 56], fp16, tag="colmax")
                cmv = colmax[:].rearrange("p (h w) -> p h w", h=114)
                nc.vector.tensor_max(cmv, v[:, :, 0:112:2], v[:, :, 1:113:2])
                nc.vector.tensor_max(cmv, cmv, v[:, :, 2:114:2])
                pool_out = ActT.alloc(hand, 64, 56, 56, 0, tag="hand")
                pov = pool_out.t[:].rearrange("p (g h w) -> p g h w", g=2, h=56)
                # pool_out layout [64, (img, 56, 56)]; write per img from packed
                povp = pool_out.t.tensor  # underlying tile
                pk = stpool.tile([128, 56 * 56], fp16, tag="poolpk")
                pkv = pk[:].rearrange("p (h w) -> p h w", h=56)
                nc.vector.tensor_max(pkv, cmv[:, 0:112:2, :], cmv[:, 1:113:2, :])
                nc.vector.tensor_max(pkv, pkv, cmv[:, 2:114:2, :])
                # unpack [p=(img,ch), hw] -> [ch, (img, hw)] via 2 sbuf DMAs
                for img in range(IMGS_PER_CORE):
                    nc.sync.dma_start(pool_out.view()[:, img, 0],
                                      pkv[img * 64:img * 64 + 64])

            cur = pool_out
            if dbg_dump(cur, "stem"):
                cur = None
            if K_UPTO == "stem":
                return

            # ---------------- stages ----------------
            for si, (sname, mid, cout, nb, stride, Hin, Hout) in enumerate(STAGES):
                if cur is None:
                    break
                with tc.tile_pool(name=f"{sname}w", bufs=8) as wpool, \
                     tc.tile_pool(name=f"{sname}a", bufs=2) as apool, \
                     tc.tile_pool(name=f"{sname}m", bufs=1) as mpool:
                    wpool._wtag = f"w{sname}"
                    cin0 = 64 if si == 0 else STAGES[si - 1][2]
                    needs = {}

                    def _addc(srcs):
                        per = {}
                        for cols, kch in srcs:
                            per[cols] = per.get(cols, 0) + kch
                        for cols, n in per.items():
                            needs[cols] = max(needs.get(cols, 0), n + 1)

                    for j in range(nb):
                        cin = cin0 if j == 0 else cout
                        _addc([(mid, _cdiv(cin, 128))])
                        _addc([(9 * mid, _cdiv(mid, 128))])
                        if j == 0:
                            _addc([(cout, _cdiv(mid, 128)),
                                   (cout, _cdiv(cin, 128))])
                        else:
                            _addc([(cout, _cdiv(mid, 128))])
                    wpool._bucket_bufs = needs
                    for j in range(nb):
                        first = j == 0
                        st = stride if first else 1
                        Hi = Hin if first else Hout
                        mid1 = ActT.alloc(mpool, mid, Hi, Hi, 1, tag="mid1")
                        # zero borders of mid1
                        mv = mid1.view()
                        nc.gpsimd.memset(
                            mv[:, :, :, 0:mid1.Hs:mid1.Hs - 1, :], 0.0)
                        nc.gpsimd.memset(
                            mv[:, :, :, :, 0:mid1.Ws:mid1.Ws - 1], 0.0)
                        mid2 = ActT.alloc(mpool, mid, Hout, Hout, 0, tag="mid2")
                        last_block = (j == nb - 1)
                        opool = hand if last_block else apool
                        out = ActT.alloc(opool, cout, Hout, Hout, 0,
                                         tag="hand" if last_block else "act")

                        pre = f"{sname}b{j}"
                        b.conv(wpool, pspool, f"W_{pre}_1", cur, mid1, mid,
                               1, 1, relu=True)
                        if dbg_dump(mid1, f"{pre}w1"):
                            return
                        b.conv(wpool, pspool, f"W_{pre}_2", mid1, mid2, mid,
                               3, st, relu=True)
                        if dbg_dump(mid2, f"{pre}w2"):
                            return
                        if first:
                            b.conv(wpool, pspool, f"W_{pre}_3", mid2, out, cout,
                                   1, 1, relu=True, dw_src=cur,
                                   dw_name=f"W_{pre}_dw", dw_stride=st)
                        else:
                            b.conv(wpool, pspool, f"W_{pre}_3", mid2, out, cout,
                                   1, 1, relu=True, res_src=cur)
                        cur = out
                        if dbg_dump(cur, f"{pre}out"):
                            return
                if dbg_dump(cur, sname):
                    cur = None
                    break
                if K_UPTO == sname:
                    return

            # ---------------- patch embed + mamba + classifier -------------
            if cur is not None:
                with tc.tile_pool(name="mw", bufs=10) as mwpool, \
                     tc.tile_pool(name="mm", bufs=6) as mpool, \
                     tc.tile_pool(name="ms", bufs=1) as spool:
                    mwpool._wtag = "wm"
                    mwpool._bucket_bufs = {256: 17, 1024: 3, 48: 5, 512: 2}
                    nbt = IMGS_PER_CORE * L_TOK  # 98

                    # patch embed: 1x1 2048->256 + bias
                    e_t = [mpool.tile([128, nbt], f32, tag="e", name="e_t") for _ in range(2)]
                    pb = b.load_bias(mwpool, "B_patch")
                    s4v = cur.view()  # [128, 2, 16, 7, 7]
                    for m in range(2):
                        ps = pspool.tile([128, nbt], f32, tag="ps")
                        for kc in range(16):
                            wt = b.load_w(mwpool, "W_patch", kc)
                            nc.tensor.matmul(
                                ps[:], wt[:, m * 128:(m + 1) * 128],
                                s4v[:, :, kc], start=(kc == 0), stop=(kc == 15))
                        b.epilogue(ps[:], e_t[m][:], pb[:, m:m + 1], False)

                    if K_UPTO == "patch" and dbg_d is not None:
                        et = spool.tile([128, 2 * nbt], f32, tag="dbg")
                        nc.vector.tensor_copy(et[:, 0:nbt], e_t[0][:])
                        nc.vector.tensor_copy(et[:, nbt:], e_t[1][:])
                        nc.gpsimd.dma_start(dbg_d, et[:])
                        return

                    for mi in range(2):
                        e_t = _mamba_block(b, nc, tc, mwpool, pspool, mpool,
                                           spool, drpool, mi, e_t, nbt)
                        if K_UPTO == f"m{mi}" and dbg_d is not None:
                            et = spool.tile([128, 2 * nbt], f32, tag="dbg")
                            nc.vector.tensor_copy(et[:, 0:nbt], e_t[0][:])
                            nc.vector.tensor_copy(et[:, nbt:], e_t[1][:])
                            nc.gpsimd.dma_start(dbg_d, et[:])
                            return

                    # mean-pool (folded 1/49 into cls weights) + classifier
                    pooled = spool.tile([128, 2 * IMGS_PER_CORE], f32, tag="pool")
                    for m in range(2):
                        pf = spool.tile([128, IMGS_PER_CORE], f32, tag="poolf")
                        ev = e_t[m][:].rearrange("p (i t) -> p i t", i=IMGS_PER_CORE)
                        nc.vector.tensor_reduce(pf[:], ev, axis=AX.X, op=OP.add)
                        nc.vector.tensor_copy(
                            pooled[:, m * IMGS_PER_CORE:(m + 1) * IMGS_PER_CORE],
                            pf[:])
                    clsb = b.load_bias(mwpool, "B_cls")
                    logits = spool.tile([128, 8 * IMGS_PER_CORE], f32, tag="logits")
                    for m in range(8):
                        ps = pspool.tile([128, IMGS_PER_CORE], f32, tag="ps")
                        for kc in range(2):
                            wt = b.load_w(mwpool, "W_cls", kc)
                            nc.tensor.matmul(
                                ps[:], wt[:, m * 128:(m + 1) * 128],
                                pooled[:, kc * IMGS_PER_CORE:(kc + 1) * IMGS_PER_CORE],
                                start=(kc == 0), stop=(kc == 1))
                        nc.scalar.activation(
                            logits[:, m * IMGS_PER_CORE:(m + 1) * IMGS_PER_CORE],
                            ps[:], AF.Identity, bias=clsb[:, m:m + 1])
                    lv = logits[:].rearrange("p (m i) -> p m i", m=8)
                    # cols 0..895 from m chunks 0..6; cols 896..999 from m=7
                    for i in range(IMGS_PER_CORE):
                        nc.sync.dma_start(
                            out_d[i, 0:896].rearrange("(m p) -> p m", m=7),
                            lv[:, 0:7, i])
                        nc.sync.dma_start(
                            out_d[i, 896:1000].unsqueeze(0).rearrange("o p -> p o"),
                            lv[0:104, 7, i].unsqueeze(1))


def _mamba_block(b, nc, tc, wpool, pspool, mpool, spool, drpool, mi, e_t, nbt):
    """One mamba block; e_t = [2 x tile [128, 98] bf16] -> returns same form."""
    W = 4  # dblk count
    TP = D_CONV - 1 + L_TOK  # 52 padded seq len

    # ---- in_proj -> xc (padded, f32), sz = silu(z) ----
    xc_pad = spool.tile([128, W * IMGS_PER_CORE * TP], f32, tag="xc")
    xv = xc_pad[:].rearrange("p (d i t) -> p d i t", d=W, i=IMGS_PER_CORE)
    nc.gpsimd.memset(xv[:, :, :, 0:D_CONV - 1], 0.0)
    sz = [mpool.tile([128, nbt], f32, tag="sz", name="sz") for _ in range(W)]
    for m in range(8):
        ps = pspool.tile([128, nbt], f32, tag="ps")
        for kc in range(2):
            wt = b.load_w(wpool, f"W_m{mi}_in", kc)
            nc.tensor.matmul(ps[:], wt[:, m * 128:(m + 1) * 128], e_t[kc][:],
                             start=(kc == 0), stop=(kc == 1))
        if m < 4:
            dst = xv[:, m, :, D_CONV - 1:]
            psv = ps[:].rearrange("p (i t) -> p i t", i=IMGS_PER_CORE)
            nc.scalar.copy(dst, psv)
        else:
            zt = mpool.tile([128, nbt], f32, tag="zt", bufs=2, name="zt")
            nc.scalar.copy(zt[:], ps[:])
            sg = mpool.tile([128, nbt], f32, tag="sg", bufs=2, name="sg")
            nc.scalar.activation(sg[:], ps[:], AF.Sigmoid)
            nc.vector.tensor_mul(sz[m - 4][:], zt[:], sg[:])

    # ---- causal conv1d + silu -> u (bf16) ----
    cw = b.load_bias(wpool, f"W_m{mi}_conv")   # [128, 16] f32
    cb = b.load_bias(wpool, f"B_m{mi}_conv")   # [128, 4]
    u = []
    for d in range(W):
        acc0 = mpool.tile([128, nbt], f32, tag="acc0", bufs=2)
        acc1 = mpool.tile([128, nbt], f32, tag="acc1", bufs=2)
        win = lambda k: xv[:, d, :, k:k + L_TOK]
        nc.vector.tensor_scalar_mul(acc0[:], win(0), cw[:, d * 4:d * 4 + 1])
        nc.vector.scalar_tensor_tensor(acc1[:], win(1), cw[:, d * 4 + 1:d * 4 + 2],
                                       acc0[:], op0=OP.mult, op1=OP.add)
        nc.vector.scalar_tensor_tensor(acc0[:], win(2), cw[:, d * 4 + 2:d * 4 + 3],
                                       acc1[:], op0=OP.mult, op1=OP.add)
        nc.vector.scalar_tensor_tensor(acc1[:], win(3), cw[:, d * 4 + 3:d * 4 + 4],
                                       acc0[:], op0=OP.mult, op1=OP.add)
        xb = mpool.tile([128, nbt], f32, tag="xb", bufs=2, name="xb")
        nc.scalar.activation(xb[:], acc1[:], AF.Identity, bias=cb[:, d:d + 1])
        sgu = mpool.tile([128, nbt], f32, tag="sg", bufs=2, name="sgu")
        nc.scalar.activation(sgu[:], xb[:], AF.Sigmoid)
        ut = mpool.tile([128, nbt], f32, tag="u")
        nc.vector.tensor_mul(ut[:], xb[:], sgu[:])
        u.append(ut)

    # ---- x_dbl = xproj @ u -> [48, 98] ----
    ps48 = pspool.tile([48, nbt], f32, tag="ps")
    for kc in range(W):
        wt = b.load_w(wpool, f"W_m{mi}_xp", kc)
        nc.tensor.matmul(ps48[:], wt[:, :48], u[kc][:],
                         start=(kc == 0), stop=(kc == 3))
    xdbl = spool.tile([48, nbt], f32, tag="xdbl")
    nc.scalar.copy(xdbl[:], ps48[:])

    # ---- dt = softplus(dt_w @ dt_r + dt_b) [f32, per dblk] ----
    dtb = b.load_bias(wpool, f"B_m{mi}_dt")
    dtw = b.load_w(wpool, f"W_m{mi}_dt", 0)  # [16, 512]
    dt = []
    for d in range(W):
        ps = pspool.tile([128, nbt], f32, tag="ps")
        nc.tensor.matmul(ps[:], dtw[:16, d * 128:(d + 1) * 128], xdbl[0:16, :],
                         start=True, stop=True)
        # softplus(x) = max(x,0) + ln(1 + exp(-|x|)); no HW softplus table
        xt = mpool.tile([128, nbt], f32, tag="dtx", bufs=2, name="xt")
        nc.vector.tensor_scalar_add(xt[:], ps[:], dtb[:, d:d + 1])
        et = mpool.tile([128, nbt], f32, tag="dte", bufs=2, name="et")
        nc.scalar.activation(et[:], xt[:], AF.Abs)
        nc.scalar.activation(et[:], et[:], AF.Exp, scale=-1.0)
        nc.scalar.activation(et[:], et[:], AF.Ln, bias=1.0)
        dtt = mpool.tile([128, nbt], f32, tag="dt", name="dtt")
        nc.vector.scalar_tensor_tensor(dtt[:], xt[:], 0.0, et[:],
                                       op0=OP.max, op1=OP.add)
        # clamped copy for the exp(dt*A) path (|A| >= 1 so dt>80 -> dA ~ 0)
        dtc = mpool.tile([128, nbt], f32, tag="dtc", bufs=5, name="dtc")
        nc.vector.tensor_scalar_min(dtc[:], dtt[:], 80.0)
        dt.append((dtt, dtc))

    # ---- B_rep / C_rep via dram-bounce flatten + ones matmul ----
    ones_t = spool.tile([1, 128], f32, tag="ones")
    nc.vector.memset(ones_t[:], 1.0)
    bc_d = drpool.tile([32, nbt], f32)
    nc.sync.dma_start(bc_d[:], xdbl[16:48, :])
    flat = spool.tile([1, 2 * D_STATE * nbt], f32, tag="flat")
    nc.sync.dma_start(flat[:],
                      bc_d[:].rearrange("n t -> (n t)").unsqueeze(0))
    reps = []
    for h in range(2):  # 0: B, 1: C
        rep = spool.tile([128, D_STATE * nbt], f32, tag=f"rep{h}")
        for q in range(4):
            ps = pspool.tile([128, 4 * nbt], f32, tag="ps")
            nc.tensor.matmul(
                ps[:], ones_t[:],
                flat[:, h * D_STATE * nbt + q * 4 * nbt:
                     h * D_STATE * nbt + (q + 1) * 4 * nbt],
                start=True, stop=True)
            nc.scalar.copy(rep[:, q * 4 * nbt:(q + 1) * 4 * nbt], ps[:])
        reps.append(rep)
    B_rep, C_rep = reps

    # ---- per-dblk: dA, dBx, scan, y ----
    SEQ = D_STATE * nbt  # 1568 per dblk
    A_sb = b.load_bias(wpool, f"A_m{mi}")  # [128, 64] f32
    D_sb = b.load_bias(wpool, f"D_m{mi}")
    y3 = []
    for d in range(W):
        tmp = mpool.tile([128, SEQ], f32, tag="datmp", bufs=2, name="datmp")
        nc.vector.tensor_tensor(
            tmp[:].rearrange("p (n t) -> p n t", n=D_STATE),
            dt[d][1][:].unsqueeze(1).broadcast_to([128, D_STATE, nbt]),
            A_sb[:, d * D_STATE:(d + 1) * D_STATE].unsqueeze(2)
                .broadcast_to([128, D_STATE, nbt]),
            op=OP.mult)
        dAd = mpool.tile([128, SEQ], f32, tag="dA", bufs=2, name="dAd")
        nc.scalar.activation(dAd[:], tmp[:], AF.Exp)
        # zero dA at sequence starts (t=0 of each (n,b) seq) -> scan resets
        nc.vector.memset(
            dAd[:].rearrange("p (s t) -> p s t", t=L_TOK)[:, :, 0:1], 0.0)
        du = mpool.tile([128, nbt], f32, tag="du", bufs=2, name="du")
        nc.vector.tensor_mul(du[:], dt[d][0][:], u[d][:])
        dBxd = mpool.tile([128, SEQ], f32, tag="dBx", bufs=2, name="dBxd")
        nc.vector.tensor_tensor(
            dBxd[:].rearrange("p (n t) -> p n t", n=D_STATE),
            du[:].unsqueeze(1).broadcast_to([128, D_STATE, nbt]),
            B_rep[:].rearrange("p (n t) -> p n t", n=D_STATE),
            op=OP.mult)
        hd = mpool.tile([128, SEQ], f32, tag="h", bufs=2, name="hd")
        nc.vector.tensor_tensor_scan(hd[:], dAd[:], dBxd[:], 0.0,
                                     op0=OP.mult, op1=OP.add)
        hc = mpool.tile([128, SEQ], f32, tag="hc", bufs=2, name="hc")
        nc.vector.tensor_mul(hc[:], hd[:], C_rep[:])
        yr = mpool.tile([128, nbt], f32, tag="yr", bufs=2, name="yr")
        nc.vector.tensor_reduce(
            yr[:], hc[:].rearrange("p (n t) -> p t n", n=D_STATE),
            axis=AX.X, op=OP.add)
        y2 = mpool.tile([128, nbt], f32, tag="y2", bufs=2, name="y2")
        nc.vector.scalar_tensor_tensor(y2[:], u[d][:], D_sb[:, d:d + 1], yr[:],
                                       op0=OP.mult, op1=OP.add)
        y3t = mpool.tile([128, nbt], f32, tag="y3", name="y3t")
        nc.vector.tensor_mul(y3t[:], y2[:], sz[d][:])
        y3.append(y3t)

    # ---- out_proj ----
    e_out = [mpool.tile([128, nbt], f32, tag="e", name="e_out") for _ in range(2)]
    for m in range(2):
        ps = pspool.tile([128, nbt], f32, tag="ps")
        for kc in range(W):
            wt = b.load_w(wpool, f"W_m{mi}_out", kc)
            nc.tensor.matmul(ps[:], wt[:, m * 128:(m + 1) * 128], y3[kc][:],
                             start=(kc == 0), stop=(kc == 3))
        nc.scalar.copy(e_out[m][:], ps[:])
    return e_out


# ----------------------------------------------------------------------------
# Input shape registry (must match _prep_weights / _im2col_stem outputs)
# ----------------------------------------------------------------------------

def _input_shapes():
    shp = {"X_imc": ((2, 147, 12544), fp16),
           "W_stem": ((2, 128, 64), fp16), "B_stem": ((128, 1), f32),
           "W_patch": ((16, 128, 256), fp16), "B_patch": ((128, 2), f32),
           "W_cls": ((2, 128, 1024), f32), "B_cls": ((128, 8), f32),
           "EYE": ((128, 128), fp16)}
    in_c = 64
    for (sname, mid, cout, nb, stride, Hin, Hout) in STAGES:
        for j in range(nb):
            pre = f"{sname}b{j}"
            cin = in_c if j == 0 else cout
            shp[f"W_{pre}_1"] = ((_cdiv(cin, 128), 128, mid), fp16)
            shp[f"B_{pre}_1"] = ((128, _cdiv(mid, 128)), f32)
            shp[f"W_{pre}_2"] = ((_cdiv(mid, 128), 128, 9 * mid), fp16)
            shp[f"B_{pre}_2"] = ((128, _cdiv(mid, 128)), f32)
            shp[f"W_{pre}_3"] = ((_cdiv(mid, 128), 128, cout), fp16)
            shp[f"B_{pre}_3"] = ((128, _cdiv(cout, 128)), f32)
            if j == 0:
                shp[f"W_{pre}_dw"] = ((_cdiv(cin, 128), 128, cout), fp16)
        in_c = cout
    for i in range(2):
        shp[f"W_m{i}_in"] = ((2, 128, 1024), f32)
        shp[f"W_m{i}_conv"] = ((128, 16), f32)
        shp[f"B_m{i}_conv"] = ((128, 4), f32)
        shp[f"W_m{i}_xp"] = ((4, 128, 48), f32)
        shp[f"W_m{i}_dt"] = ((1, 16, 512), f32)
        shp[f"B_m{i}_dt"] = ((128, 4), f32)
        shp[f"A_m{i}"] = ((128, 64), f32)
        shp[f"D_m{i}"] = ((128, 4), f32)
        shp[f"W_m{i}_out"] = ((4, 128, 256), f32)
    return shp


_INPUT_SHAPES = _input_shapes()

_COMPILED = None
_DEBUG_TAP = None  # set to (upto_name, shape) before calling kernel for taps


def _get_compiled():
    global _COMPILED
    if _COMPILED is None:
        nc = bacc.Bacc("TRN2", target_bir_lowering=False, debug=False,
                       num_devices=N_CORES)
        build(nc, debug_tap=_DEBUG_TAP)
        nc.compile()
        _COMPILED = nc
    return _COMPILED


def kernel(x, params):
    x = np.asarray(x, np.float32)
    assert x.shape == (16, 3, 224, 224)
    nc = _get_compiled()
    wins = _prep_weights(params)
    in_maps = []
    for c in range(N_CORES):
        m = dict(wins)
        m["X_imc"] = _im2col_stem(x[2 * c:2 * c + 2])
        in_maps.append(m)
    res = run_bass_kernel_spmd(nc, in_maps, core_ids=list(range(N_CORES)),
                               trace=bool(os.environ.get("K_TRACE")))
    if _DEBUG_TAP is not None:
        return res, np.concatenate([r["dbg"][None] for r in res.results], 0)
    out = np.concatenate([r["out"] for r in res.results], 0)
    return out.astype(np.float32)
